# revision 57
# baseline (speedup 1.0000x reference)
"""Multi-head causal attention (B=4, S=2048, C=1024, H=16, D=64) on 8 trn2 cores.

Sharding: batch x head-half (tensor parallel over heads). Core c = (batch
b = c//2, head-half hh = c%2, heads hh*8..hh*8+8). Every core runs an
IDENTICAL program on different data: project K^T/V/Q^T for its 8 heads over
the full sequence (no replicated compute), run causal attention for all
2048 query rows, then a PARTIAL output projection over its 512 hd rows
(out_part = O^T.T @ Wp[own rows] (+bp on even cores)). The host sums the
two partial outputs per batch (the unshard step for reduction-sharded
outputs) -- no cross-core communication on device.

Attention structure: 256-row q-blocks paired as (2i, 2i+1), i=0..3,
processed at N=512 (shared key tiles) + N=256 (the odd block's 2 extra key
tiles); block-causality via per-tile diagonal masks (multiply post-exp).
Scores use K=64 contraction; the two heads of a head-pair are emitted
back-to-back with disjoint PE row groups (partitions 0:64 / 64:128) so the
matmuls overlap on hardware. exp on ACT (scale 1/8, no max subtraction:
scores ~ N(0,1)); softmax denominator rides the PV matmul as a ones column
of V (row 64 of the PSUM output).

Per-core pipeline: P1 DMA + early K/Q/V projection (first 512 keys/rows);
P2 per (pair, head-pair, key-tile) items with remaining projections and P3
(output projection) interleaved as PE fillers; P3 tail for the last pair.
"""

import numpy as np

B, S, C, H, D = 4, 2048, 1024, 16, 64
HD = H * D
HPC = H // 2        # heads per core = 8
QB = 256            # q block width
CK = C // 128       # contraction chunks over C
NCORES = 8
KRUNB = [4, 8, 12, 16]   # total key tiles for pair i (= 4i+4)

_CACHE = {}


def _build_nc():
    import concourse.bacc as bacc
    import concourse.mybir as mybir
    import concourse.tile as tile

    dt = mybir.dt
    F32, BF = dt.float32, dt.bfloat16
    EXP = mybir.ActivationFunctionType.Exp

    nc = bacc.Bacc(num_swdge_queues=4)
    xt_d = nc.declare_dram_parameter("xt", [C, S], BF, isOutput=False)
    wk_d = nc.declare_dram_parameter("wk", [C, 512], BF, isOutput=False)
    wv_d = nc.declare_dram_parameter("wv", [C, 512], BF, isOutput=False)
    wq_d = nc.declare_dram_parameter("wq", [C, 512], BF, isOutput=False)
    wp_d = nc.declare_dram_parameter("wp", [512, C], BF, isOutput=False)
    bp_d = nc.declare_dram_parameter("bp", [1, C], F32, isOutput=False)
    mk_d = nc.declare_dram_parameter("masks", [16, 2, 128, QB], BF, isOutput=False)
    out_d = nc.declare_dram_parameter("out", [S, C], F32, isOutput=True)

    xt_r = xt_d[:].rearrange("(i p) s -> p i s", p=128)
    wk_r = wk_d[:].rearrange("(i p) n -> p i n", p=128)
    wv_r = wv_d[:].rearrange("(i p) n -> p i n", p=128)
    wq_r = wq_d[:].rearrange("(i p) n -> p i n", p=128)
    wp_r = wp_d[:].rearrange("(i p) n -> p i n", p=128)
    mk_r = mk_d[:].rearrange("s d p q -> p (s d) q")

    with tile.TileContext(nc) as tc:
        with (
            tc.tile_pool(name="persist", bufs=1) as PP,
            tc.tile_pool(name="psum", bufs=1, space="PSUM") as PS,
            tc.tile_pool(name="outp", bufs=5) as OP,
        ):
            # persistent tensors
            kt_sb = PP.tile([128, 4, S], BF, tag="kt")          # K^T
            qt_sb = PP.tile([128, 4, S], BF, tag="qt")          # Q^T
            ot_sb = PP.tile([128, 4, S], BF, tag="ot")          # O^T
            v_sb = PP.tile([128, 16, HPC, D + 1], BF, tag="v")  # V + ones col
            mask_sb = PP.tile([128, 32, QB], BF, tag="mask")    # 16 tiles x 2 heads
            bb_sb = PP.tile([128, C], F32, tag="bb")
            bp1_sb = PP.tile([1, C], F32, tag="bp1")
            wp_sb = PP.tile([128, 4, C], BF, tag="wp")
            wk_sb = PP.tile([128, CK, 512], BF, tag="wk")
            wq_sb = PP.tile([128, CK, 512], BF, tag="wq")
            wv_sb = PP.tile([128, CK, 512], BF, tag="wv")
            xt_sb = PP.tile([128, CK, S], BF, tag="xt")

            # ---- input DMAs first: bf16 on host (half the bytes, no cast)
            # so they can ride the two parallel HWDGE queues (SP + ACT)
            # alongside the SWDGE gpsimd queue ----
            # chunk-granular lead DMAs: kproj(0)'s c-loop can start
            # accumulating once the first chunks land
            for h in range(4):
                cs = slice(2 * h, 2 * h + 2)
                nc.scalar.dma_start(wk_sb[:, cs, :], wk_r[:, cs, :])
                nc.sync.dma_start(xt_sb[:, cs, 0:512], xt_r[:, cs, 0:512])
            nc.scalar.dma_start(wq_sb[:], wq_r)
            nc.gpsimd.dma_start(wv_sb[:], wv_r)
            for nt in range(1, 4):
                sl = slice(nt * 512, nt * 512 + 512)
                eng = (nc.sync, nc.scalar, nc.gpsimd)[nt - 1]
                eng.dma_start(xt_sb[:, :, sl], xt_r[:, :, sl])
            nc.sync.dma_start(mask_sb[:], mk_r)
            nc.gpsimd.dma_start(wp_sb[:], wp_r)

            # ones column of V (softmax denominator via PV matmul); bias
            # broadcast for P3 -- none of this gates the projections
            for tt in range(16):
                nc.gpsimd.memset(v_sb[:, tt, :, D : D + 1], 1.0)
            nc.sync.dma_start(bp1_sb[:], bp_d[:])
            nc.gpsimd.partition_broadcast(bb_sb[:], bp1_sb[:])

            # PE warm-up while the first DMAs land
            warm = PP.tile([128, 512], BF, tag="warm")
            nc.vector.memset(warm[:], 0.0)
            wps = PS.tile([128, 512], F32, tag="proj", bufs=2, name="warmps")
            for _ in range(6):
                nc.tensor.matmul(wps[:], warm[:, 0:128], warm[:],
                                 start=True, stop=True)

            # ---- projection units ----
            def kproj(nt):
                """K^T for keys [512*nt, 512*nt+512), all 4 head pairs."""
                sl = slice(nt * 512, nt * 512 + 512)
                for hp in range(4):
                    ps = PS.tile([128, 512], F32, tag="proj", bufs=2, name="psk")
                    for c in range(CK):
                        nc.tensor.matmul(
                            ps[:],
                            wk_sb[:, c, hp * 128 : hp * 128 + 128],
                            xt_sb[:, c, sl],
                            start=(c == 0), stop=(c == CK - 1),
                        )
                    nc.vector.tensor_copy(kt_sb[:, hp, sl], ps[:])

            def qproj(nt):
                sl = slice(nt * 512, nt * 512 + 512)
                for hp in range(4):
                    ps = PS.tile([128, 512], F32, tag="proj", bufs=2, name="psq")
                    for c in range(CK):
                        nc.tensor.matmul(
                            ps[:],
                            wq_sb[:, c, hp * 128 : hp * 128 + 128],
                            xt_sb[:, c, sl],
                            start=(c == 0), stop=(c == CK - 1),
                        )
                    nc.vector.tensor_copy(qt_sb[:, hp, sl], ps[:])

            def vproj(tt):
                """V rows for keys [128*tt, 128*tt+128), all 8 heads."""
                ps = PS.tile([128, 512], F32, tag="proj", bufs=2, name="psv")
                for c in range(CK):
                    nc.tensor.matmul(
                        ps[:],
                        xt_sb[:, c, tt * 128 : tt * 128 + 128],
                        wv_sb[:, c, :],
                        start=(c == 0), stop=(c == CK - 1),
                    )
                nc.vector.tensor_copy(
                    v_sb[:, tt, :, 0:D],
                    ps[:].rearrange("p (a b) -> p a b", b=D),
                )

            # ---- P1 lead-in: what pair 0 needs ----
            kproj(0)
            qproj(0)
            for tt in range(4):
                vproj(tt)

            # ---------------- P2 + interleaved fillers ----------------
            with (
                tc.tile_pool(name="ptp", bufs=5) as PTP,
                tc.tile_pool(name="smallp", bufs=2) as SMP,
            ):
                po_state = {}

                def emit_scores(i, hp, kt):
                    """Both heads of head-pair hp, key tile kt, q pair i.

                    The two heads' score matmuls use disjoint PE row groups
                    (K=64, partitions 0:64 / 64:128) and are emitted
                    back-to-back so they overlap on hardware; separate PSUM
                    tiles + per-head exp keep the bank budget at 8.
                    """
                    shared = kt < 4 * i + 2
                    qsl = (slice(i * 512, i * 512 + 512) if shared
                           else slice(i * 512 + QB, i * 512 + 512))
                    ksl = slice(kt * 128, kt * 128 + 128)
                    csl = slice(0, 512) if shared else slice(QB, 512)
                    pss = [PS.tile([128, 512], F32, tag="pss", bufs=2,
                                   name=f"pss{j}") for j in range(2)]
                    pt = PTP.tile([128, 2, 512], BF, tag="pt")
                    for j in range(2):
                        hr = j * 64
                        nc.tensor.matmul(
                            pss[j][:, csl],
                            kt_sb[hr : hr + 64, hp, ksl],
                            qt_sb[hr : hr + 64, hp, qsl],
                            start=True, stop=True,
                        )
                    for j in range(2):
                        nc.scalar.activation(pt[:, j, csl], pss[j][:, csl],
                                             EXP, scale=float(D) ** -0.5)
                    # diagonal masks: kt in {4i, 4i+1} masks block a (cols
                    # 0:256); kt in {4i+2, 4i+3} masks block b (cols 256:512)
                    d = kt - 4 * i
                    if d >= 0:
                        coff = 0 if d < 2 else QB
                        m = i * 8 + d * 2
                        nc.vector.tensor_mul(
                            pt[:, :, coff : coff + QB],
                            pt[:, :, coff : coff + QB],
                            mask_sb[:, m : m + 2, :],
                        )
                    return pt

                def emit_pv(i, hp, kt, pt):
                    shared = kt < 4 * i + 2
                    last = 4 * i + 4
                    if kt == 0:
                        po_state[(i, hp)] = PS.tile(
                            [128, 2, 512], F32, tag="pso", bufs=2,
                            name=f"po{i}_{hp}",
                        )
                    po = po_state[(i, hp)]
                    for j in range(2):
                        if shared:
                            nc.tensor.matmul(
                                po[0:65, j, :], v_sb[:, kt, 2 * hp + j, :],
                                pt[:, j, :],
                                start=(kt == 0), stop=(kt == last - 1),
                                skip_group_check=True,
                            )
                        else:
                            nc.tensor.matmul(
                                po[0:65, j, QB:512], v_sb[:, kt, 2 * hp + j, :],
                                pt[:, j, QB:512],
                                start=False, stop=(kt == last - 1),
                                skip_group_check=True,
                            )
                    if kt == last - 1:
                        qsl = slice(i * 512, i * 512 + 512)
                        rc = SMP.tile([128, 2, 512], F32, tag="recip")
                        rc2 = SMP.tile([128, 2, 512], F32, tag="recip2")
                        rb = SMP.tile([128, 2, 512], F32, tag="rbc")
                        # reciprocal_approx_fast is a custom DVE ucode op and
                        # cannot read PSUM on hardware -- stage via SBUF
                        nc.vector.tensor_copy(rc[0:1, :, :], po[64:65, :, :])
                        nc.vector.reciprocal_approx_fast(rc2[0:1, :, :],
                                                         rc[0:1, :, :])
                        nc.gpsimd.partition_broadcast(rb[0:64, :, :],
                                                      rc2[0:1, :, :])
                        for j in range(2):
                            nc.vector.tensor_mul(
                                ot_sb[j * 64 : j * 64 + 64, hp, qsl],
                                po[0:64, j, :], rb[0:64, j, :],
                            )
                        del po_state[(i, hp)]

                def emit_p3(rt, cb, hcs=(0, 1, 2, 3), final=True):
                    """Output-projection tile: rows [128*rt, 128*rt+128).

                    hcs selects the hd chunks to accumulate this call; with
                    final=False the partial (+bias) is parked in the ob SBUF
                    tile, a later call adds the remaining chunks and DMAs.
                    """
                    qsl = slice(rt * 128, rt * 128 + 128)
                    csl = slice(cb * 512, cb * 512 + 512)
                    first = cb == 0 and (0 in hcs)
                    if rt not in emit_p3.ob:
                        emit_p3.ob[rt] = OP.tile([128, C], F32, tag="ob",
                                                 name=f"ob{rt}")
                    ob = emit_p3.ob[rt]
                    ps = PS.tile([128, 512], F32, tag="proj", bufs=2, name="psf")
                    for n_, hc in enumerate(hcs):
                        nc.tensor.matmul(
                            ps[:],
                            ot_sb[:, hc, qsl],
                            wp_sb[:, hc, csl],
                            start=(n_ == 0), stop=(n_ == len(hcs) - 1),
                        )
                    if 0 in hcs:
                        nc.vector.tensor_add(ob[:, csl], ps[:], bb_sb[:, csl])
                    else:
                        nc.vector.tensor_add(ob[:, csl], ob[:, csl], ps[:])
                    if final:
                        eng = nc.sync if (rt + cb) % 2 == 0 else nc.gpsimd
                        eng.dma_start(out_d[qsl, csl], ob[:, csl])
                        if cb == 1:
                            del emit_p3.ob[rt]
                emit_p3.ob = {}

                def emit_p3_partial(rt, cb):
                    emit_p3(rt, cb, hcs=(0, 1, 2), final=False)

                def emit_p3_final(rt, cb):
                    emit_p3(rt, cb, hcs=(3,), final=True)

                # items and fillers
                items = [(i, hp, kt)
                         for i in range(4)
                         for hp in range(4)
                         for kt in range(KRUNB[i])]
                # filler units become eligible at item index n (first field)
                fills = []
                # during pair 0 (items 0..15): K nt1, Q nt1, V kt4..7
                fills += [(0 + 2 * t, "v", (4 + t,)) for t in range(4)]
                fills += [(8, "k", (1,)), (12, "q", (1,))]
                # during pair 1 (16..47): K nt2, Q nt2, V kt8..11, P3 pair 0
                fills += [(16 + 4 * t, "v", (8 + t,)) for t in range(4)]
                fills += [(20, "k", (2,)), (28, "q", (2,))]
                fills += [(34 + 2 * t, "p3", (t // 2, t % 2)) for t in range(8)]
                # during pair 2 (48..95): K nt3, Q nt3, V kt12..15, P3 pair 1
                fills += [(48 + 6 * t, "v", (12 + t,)) for t in range(4)]
                fills += [(54, "k", (3,)), (66, "q", (3,))]
                fills += [(74 + 3 * t, "p3", (4 + t // 2, t % 2)) for t in range(8)]
                # during pair 3 (96..159): P3 pair 2, except 5 units held
                # back as PE filler for the pend-drain dependency chain;
                # pair-3 P3 partials (hd chunks 0-2, ready once head-pairs
                # 0-2 have normalized) slot in near the end
                fills += [(100 + 12 * t, "p3", (8 + t // 2, t % 2)) for t in range(3)]
                # eligibility: normalize(3, hp=2) is emitted with the pv
                # popped at item 147 (pend depth 4) -- partials after that
                fills += [(148 + t, "p3p", (12 + t // 2, t % 2)) for t in range(8)]
                drain_fills = [("p3", (8 + t // 2, t % 2)) for t in range(3, 8)]
                fills.sort(key=lambda f: f[0])
                fns = {"k": kproj, "q": qproj, "v": vproj, "p3": emit_p3,
                       "p3p": emit_p3_partial}

                pend = []
                for n, it in enumerate(items):
                    pt = emit_scores(*it)
                    pend.append((it, pt))
                    if len(pend) > 4:
                        old = pend.pop(0)
                        emit_pv(*old[0], old[1])
                    while fills and fills[0][0] <= n:
                        _, kind, args = fills.pop(0)
                        fns[kind](*args)
                for _, kind, args in fills:
                    fns[kind](*args)
                # drain: the final PVs block on the exp chain and the last
                # normalize blocks P3; keep real P3 work plus warm-keeper
                # matmuls behind them so the PE neither idles nor cools down
                for old in pend:
                    emit_pv(*old[0], old[1])
                for kind, args in drain_fills:
                    fns[kind](*args)
                for w in range(6):
                    wps2 = PS.tile([128, 512], F32, tag="proj", bufs=2,
                                   name=f"warm2_{w}")
                    nc.tensor.matmul(wps2[:], warm[:, 0:128], warm[:],
                                     start=True, stop=True)

                # P3 tail: pair 3 rows, final hd chunk only
                for t in range(8):
                    emit_p3_final(12 + t // 2, t % 2)

    nc.finalize()
    return nc


def _get_runner():
    """Compile once; return fn(in_maps) -> list[dict] using a cached jax jit."""
    if "runner" in _CACHE:
        return _CACHE["runner"]
    import jax
    import concourse.mybir as mybir
    from concourse import bass2jax as b2j
    from jax.experimental.shard_map import shard_map
    from jax.sharding import Mesh, PartitionSpec

    nc = _build_nc()
    b2j.install_neuronx_cc_hook()

    partition_name = nc.partition_id_tensor.name if nc.partition_id_tensor else None
    in_names, out_names, out_avals, zero_outs = [], [], [], []
    for alloc in nc.m.functions[0].allocations:
        if not isinstance(alloc, mybir.MemoryLocationSet):
            continue
        name = alloc.memorylocations[0].name
        if alloc.kind == "ExternalInput":
            if name != partition_name:
                in_names.append(name)
        elif alloc.kind == "ExternalOutput":
            shape = tuple(alloc.tensor_shape)
            dtype = mybir.dt.np(alloc.dtype)
            out_names.append(name)
            out_avals.append(jax.core.ShapedArray(shape, dtype))
            zero_outs.append(np.zeros(shape, dtype))
    n_params = len(in_names)
    n_outs = len(out_avals)
    in_names = in_names + out_names
    if partition_name is not None:
        in_names.append(partition_name)
    donate = tuple(range(n_params, n_params + n_outs))

    def _body(*args):
        operands = list(args)
        if partition_name is not None:
            operands.append(b2j.partition_id_tensor())
        outs = b2j._bass_exec_p.bind(
            *operands,
            out_avals=tuple(out_avals),
            in_names=tuple(in_names),
            out_names=tuple(out_names),
            lowering_input_output_aliases=(),
            sim_require_finite=True,
            sim_require_nnan=True,
            nc=nc,
        )
        return tuple(outs)

    try:
        devices = jax.devices("axon")[:NCORES]
    except RuntimeError:
        devices = jax.devices()[:NCORES]
    mesh = Mesh(np.asarray(devices), ("core",))
    in_specs = (PartitionSpec("core"),) * (n_params + n_outs)
    out_specs = (PartitionSpec("core"),) * n_outs
    sharded = jax.jit(
        shard_map(_body, mesh=mesh, in_specs=in_specs, out_specs=out_specs,
                  check_rep=False),
        donate_argnums=donate,
        keep_unused=True,
    )
    _CACHE["mesh"] = mesh
    _CACHE["sharded_nodonate"] = jax.jit(
        shard_map(_body, mesh=mesh, in_specs=in_specs, out_specs=out_specs,
                  check_rep=False),
        keep_unused=True,
    )

    def runner(in_maps):
        per_core = [[np.asarray(m[nm]) for nm in in_names[:n_params]] for m in in_maps]
        concat_in = [
            np.concatenate([per_core[c][i] for c in range(NCORES)], axis=0)
            for i in range(n_params)
        ]
        concat_zeros = [
            np.zeros((NCORES * z.shape[0], *z.shape[1:]), z.dtype) for z in zero_outs
        ]
        out_arrs = sharded(*concat_in, *concat_zeros)
        return [
            {
                nm: np.asarray(out_arrs[i]).reshape(NCORES, *out_avals[i].shape)[c]
                for i, nm in enumerate(out_names)
            }
            for c in range(NCORES)
        ]

    _CACHE["nc"] = nc
    _CACHE["runner"] = runner
    return runner


def make_in_maps(x, Wq, Wk, Wv, Wp, bp):
    import ml_dtypes

    bf16 = ml_dtypes.bfloat16
    x = np.asarray(x, np.float32)
    wq_h = np.asarray(Wq, bf16).transpose(1, 0, 2).reshape(C, HD)
    wk_h = np.asarray(Wk, bf16).transpose(1, 0, 2).reshape(C, HD)
    wv_h = np.asarray(Wv, bf16).transpose(1, 0, 2).reshape(C, HD)
    wp_h = np.asarray(Wp, bf16)
    bp1 = np.asarray(bp, np.float32).reshape(1, C)

    # diagonal masks, shared by all cores: pair i, tiles for kt = 4i+d
    mk = np.zeros((16, 2, 128, QB), bf16)
    for i in range(4):
        for d in range(4):
            kt = 4 * i + d
            blk = 2 * i + (0 if d < 2 else 1)       # 256-block index
            qabs = blk * QB + np.arange(QB)[None, :]
            kabs = kt * 128 + np.arange(128)[:, None]
            mk[i * 4 + d, :, :, :] = (kabs <= qabs).astype(bf16)[None]

    in_maps = []
    xt_b = [np.ascontiguousarray(x[b].T.astype(bf16)) for b in range(B)]
    for core in range(NCORES):
        b, hh = core // 2, core % 2
        hsl = slice(hh * 512, hh * 512 + 512)
        in_maps.append({
            "xt": xt_b[b],
            "wq": np.ascontiguousarray(wq_h[:, hsl]),
            "wk": np.ascontiguousarray(wk_h[:, hsl]),
            "wv": np.ascontiguousarray(wv_h[:, hsl]),
            "wp": np.ascontiguousarray(wp_h[hsl, :]),
            "bp": bp1 if hh == 0 else np.zeros_like(bp1),
            "masks": mk,
        })
    return in_maps, None


def assemble(results, _unused=None):
    out = np.empty((B, S, C), np.float32)
    for b in range(B):
        out[b] = results[2 * b]["out"]
        out[b] += results[2 * b + 1]["out"]
    return out


def kernel(x, Wq, Wk, Wv, Wp, bp):
    in_maps, aux = make_in_maps(x, Wq, Wk, Wv, Wp, bp)
    runner = _get_runner()
    results = runner(in_maps)
    return assemble(results, aux)
